# revision 31
# baseline (speedup 1.0000x reference)
"""Trainium2 Bass kernel: single-head causal attention, data-parallel over batch.

Per core (one batch element):
    Q = x @ w_q; K = x @ w_k; V = (x @ w_v1) @ w_v2
    out = softmax_causal(Q K^T / sqrt(64)) @ V

Sharding: batch 8 -> one element per NeuronCore, weights replicated.

Design notes (v6):
- Host prep: x^T bf16 [E, S]; w_q (scale folded) and w_k fused into one
  [128, NE, 128] lhsT tile so K^T and Q^T come from a single M=128 matmul
  per E-chunk.  w_v2 duplicated into both partition halves.
- Low-rank reassociation: attn @ V = (attn @ Vp) @ w_v2 with Vp = x@w_v1.
- Scores computed transposed (S^T = K Q^T) so exp'd P^T is in numerator
  lhsT layout; ones column on Vp makes the softmax denominator for free.
- 2x PE via array row-tiling: K^T/Q^T are materialized in both partition
  halves (qkt native + qk2 swapped by two DVE crossbar partition-shift
  copies - no DMA, so nothing queues behind the x^T bulk).  Score strips
  run in pairs (rows 0:64 / 64:128 into the two banks of one PSUM tile);
  out-GEMM tiles (t, t+2) pair the same way via a split numerator evac.
  Vp projections for blocks 2,3 pair via column tiling.
- Single unified 6-bank PSUM ring (3 x [128,2,512]) for projections,
  strip pairs and out tiles: one FD-1024 ACT exp per strip pair, one
  FD-1024 DVE/ACT evac per out tile.
- Output bf16 (host upcasts); one batched out DMA per group on the sync
  ring, per-tile across all rings for the last group.
- Group order 0,1,2,3 dovetails with x^T arrival; group g+1's
  projections and strips are emitted inside period g.
"""

import os
import sys

import numpy as np

for _p in ("/opt/trn_rl_repo", "/root/.axon_site/_ro/trn_rl_repo"):
    if os.path.isdir(_p) and _p not in sys.path:
        sys.path.insert(0, _p)
os.environ.setdefault("MYCRO_LOCAL_CACHE", "1")

import ml_dtypes  # noqa: E402
import concourse.bass as bass  # noqa: E402
import concourse.mybir as mybir  # noqa: E402
import concourse.tile as tile  # noqa: E402
from concourse import bacc  # noqa: E402
from concourse import bass_utils  # noqa: E402
from concourse.masks import make_identity, make_upper_triangular  # noqa: E402

F32 = mybir.dt.float32
BF16 = mybir.dt.bfloat16

B, S, E, D = 8, 2048, 1024, 64
P = 128
NS = S // P       # 16 s/q tiles
NE = E // P       # 8 E-chunks (projection contraction)
QG = 512          # q-group width
NQG = S // QG     # 4 q-groups
GT = QG // P      # 4 q-tiles per group
SCALE = D ** -0.5
EXP_FN = mybir.ActivationFunctionType.Exp
COPY_FN = mybir.ActivationFunctionType.Copy

ORD = [0, 1, 2, 3]  # group processing order (dovetails with x^T DMA arrival)


def build_kernel(nc):
    x_t = nc.dram_tensor("x_t", (E, S), BF16, kind="ExternalInput").ap()
    # wqk[p, c*128+m]: m 0:64 = w_k[c*128+p, m], m 64:128 = w_q*scale
    wqk = nc.dram_tensor("wqk", (P, NE * P), BF16, kind="ExternalInput").ap()
    w_v1 = nc.dram_tensor("w_v1", (P, NE * D), BF16, kind="ExternalInput").ap()
    w_v2 = nc.dram_tensor("w_v2", (D, E), BF16, kind="ExternalInput").ap()
    out = nc.dram_tensor("out", (S, E), BF16, kind="ExternalOutput").ap()

    with tile.TileContext(nc) as tc:
        _body(tc, nc, x_t, wqk, w_v1, w_v2, out)


def _body(tc, nc, x_t, wqk, w_v1, w_v2, out):
    from contextlib import ExitStack

    with ExitStack() as ctx:
        const = ctx.enter_context(tc.tile_pool(name="const", bufs=1))
        big = ctx.enter_context(tc.tile_pool(name="big", bufs=1))
        ptp = ctx.enter_context(tc.tile_pool(name="ptp", bufs=14))
        outp = ctx.enter_context(tc.tile_pool(name="outp", bufs=2))
        small = ctx.enter_context(tc.tile_pool(name="small", bufs=2))
        psS = ctx.enter_context(tc.tile_pool(name="psS", bufs=3, space="PSUM"))
        psN = ctx.enter_context(tc.tile_pool(name="psN", bufs=2, space="PSUM"))

        # ---------------- input DMAs ----------------
        xT = big.tile([P, NE, S], BF16, tag="xT")  # xT[p, c, s] = x[s, c*128+p]
        xtv = x_t.rearrange("(c p) s -> p c s", p=P)
        wqk_sb = const.tile([P, NE, P], BF16, tag="wqk")
        wv1_sb = const.tile([P, NE, D], BF16, tag="wv1")
        wv2_sb = const.tile([P, E], BF16, tag="wv2")  # both halves hold w_v2
        wqkv = wqk.rearrange("p (c m) -> p c m", m=P)
        nc.scalar.dma_start(wqk_sb[:, 0:2, :], wqkv[:, 0:2, :])
        nc.scalar.dma_start(wqk_sb[:, 2:NE, :], wqkv[:, 2:NE, :])
        nc.scalar.dma_start(wv1_sb[:, :, :],
                            w_v1.rearrange("p (c d) -> p c d", d=D))
        # x^T: group 0 in 2-chunk pieces, groups 1-3 in halves, all on the
        # sync/gpsimd rings (scalar ring carries only the small weights)
        for k in range(4):
            eng = (nc.sync, nc.gpsimd)[k % 2]
            eng.dma_start(xT[:, 2 * k:2 * k + 2, 0:QG],
                          xtv[:, 2 * k:2 * k + 2, 0:QG])
        for ng in (1, 2, 3):
            sl = slice(ng * QG, (ng + 1) * QG)
            nc.sync.dma_start(xT[:, 0:4, sl], xtv[:, 0:4, sl])
            nc.gpsimd.dma_start(xT[:, 4:NE, sl], xtv[:, 4:NE, sl])
        nc.scalar.dma_start(wv2_sb[0:D, :], w_v2)
        nc.scalar.dma_start(wv2_sb[D:P, :], w_v2)

        # ---------------- SBUF staging ----------------
        # qkt: K^T rows 0:64, Q^T rows 64:128 (native PSUM layout)
        qkt = big.tile([P, S], BF16, tag="qkt")
        # qk2: swapped copy - Q^T rows 0:64, K^T rows 64:128 (DVE crossbar)
        qk2 = big.tile([P, S], BF16, tag="qk2")
        # vpt: Vp^T blocks; rows 0:64 hold even blocks, 64:128 odd blocks
        vpt = big.tile([P, S], BF16, tag="vpt")
        vp_sb = big.tile([P, NS, D + 1], BF16, tag="vp")
        # num: rows 0:64 cols 0:256, rows 64:128 cols 256:512
        num_sb = big.tile([P, NQG, QG], BF16, tag="num")
        ident = const.tile([P, P], BF16, tag="ident")
        ident4 = const.tile([GT, GT], F32, tag="ident4")
        tri = const.tile([P, P], BF16, tag="tri")
        nc.vector.memset(vp_sb[:, :, D], 1.0)

        def qk_pass(ng):
            """Fused [w_k|w_q] projection: one M=128 matmul per E-chunk."""
            sl = slice(ng * QG, (ng + 1) * QG)
            ps = psS.tile([P, 2, QG], F32, tag="psS")
            for ec in range(NE):
                nc.tensor.matmul(ps[:, 0, :], wqk_sb[:, ec, :],
                                 xT[:, ec, sl],
                                 start=(ec == 0), stop=(ec == NE - 1))
            nc.scalar.activation(qkt[:, sl], ps[:, 0, :], COPY_FN)
            # swapped copy via DVE crossbar partition shifts (no DMA)
            nc.vector.tensor_copy(qk2[0:D, sl], qkt[D:P, sl])
            nc.vector.tensor_copy(qk2[D:P, sl], qkt[0:D, sl])

        def vp_pair(ngA, ngB):
            """Vp^T for two blocks via column-tiled concurrent matmuls."""
            slA = slice(ngA * QG, (ngA + 1) * QG)
            slB = slice(ngB * QG, (ngB + 1) * QG)
            ps = psS.tile([P, 2, QG], F32, tag="psS")
            for ec in range(NE):
                nc.tensor.matmul(ps[0:D, 0, :], wv1_sb[:, ec, :],
                                 xT[:, ec, slA],
                                 start=(ec == 0), stop=(ec == NE - 1),
                                 tile_position=(0, 0), skip_group_check=True)
                nc.tensor.matmul(ps[D:P, 0, :], wv1_sb[:, ec, :],
                                 xT[:, ec, slB],
                                 start=(ec == 0), stop=(ec == NE - 1),
                                 tile_position=(0, D), skip_group_check=True)
            nc.vector.tensor_copy(vpt[0:D, slA], ps[0:D, 0, :])
            nc.vector.tensor_copy(vpt[D:P, slB], ps[D:P, 0, :])

        def vp_solo(ng):
            """Vp^T block ng -> vpt rows 0:64 (even ng) / 64:128 (odd)."""
            sl = slice(ng * QG, (ng + 1) * QG)
            ps = psS.tile([P, 2, QG], F32, tag="psS")
            lo, hi = (0, D) if ng % 2 == 0 else (D, P)
            for ec in range(NE):
                nc.tensor.matmul(ps[lo:hi, 0, :], wv1_sb[:, ec, :],
                                 xT[:, ec, sl],
                                 start=(ec == 0), stop=(ec == NE - 1),
                                 tile_position=(0, lo),
                                 skip_group_check=True)
            nc.vector.tensor_copy(vpt[lo:hi, sl], ps[lo:hi, 0, :])

        def vp_transp(ng):
            """Vp^T block -> four [128,64] vp tiles via PE transpose."""
            half = ng % 2
            lo, hi = (0, D) if half == 0 else (D, P)
            pst = psS.tile([P, GT, D], BF16, tag="psS")
            for i in range(GT):
                st = ng * GT + i
                nc.tensor.transpose(
                    pst[:, i, :], vpt[lo:hi, st * P:(st + 1) * P],
                    ident[lo:hi, lo:hi],
                    tile_position=(lo, 0) if half else None)
            nc.vector.tensor_copy(vp_sb[:, ng * GT:(ng + 1) * GT, 0:D],
                                  pst[:, :, :])

        def strip_pair(g, j0):
            """Score strips j0 (rows 0:64) and j0+1 (rows 64:128),
            concurrently into the two banks of one psS tile; one FD-1024
            exp (stale-PSUM cols in diagonal pairs are exp'd, never read)."""
            ps = psS.tile([P, 2, QG], F32, tag="psS")
            pt = ptp.tile([P, 2, QG], BF16, tag="pt")
            ents = []
            los = []
            for k in range(2):
                j = j0 + k
                dt = j - g * GT
                lo = dt * P if 0 < dt < GT else 0
                los.append((k, j, dt, lo))
                if k == 0:
                    nc.tensor.matmul(
                        ps[:, 0, lo:QG],
                        qkt[0:D, j * P:(j + 1) * P],
                        qk2[0:D, g * QG + lo:(g + 1) * QG],
                        start=True, stop=True,
                        tile_position=(0, 0), skip_group_check=True)
                else:
                    nc.tensor.matmul(
                        ps[:, 1, lo:QG],
                        qk2[D:P, j * P:(j + 1) * P],
                        qkt[D:P, g * QG + lo:(g + 1) * QG],
                        start=True, stop=True,
                        tile_position=(D, 0), skip_group_check=True)
            lo0 = los[0][3]
            psf = ps.rearrange("p a b -> p (a b)")
            ptf = pt.rearrange("p a b -> p (a b)")
            nc.scalar.activation(ptf[:, lo0:2 * QG], psf[:, lo0:2 * QG],
                                 EXP_FN)
            for (k, j, dt, lo) in los:
                if 0 <= dt < GT:
                    # alternate mask engine to spread semaphore load
                    eng = nc.gpsimd if k == 0 else nc.vector
                    eng.tensor_mul(
                        pt[:, k, dt * P:(dt + 1) * P],
                        pt[:, k, dt * P:(dt + 1) * P],
                        tri[:, :])
                ents.append((j, pt[:, k, lo:QG], lo))
            return ents

        def epilogue(g, psn):
            """Denominator -> per-partition recip; numerator -> bf16 split:
            cols 0:256 at rows 0:64, cols 256:512 at rows 64:128."""
            d_sb = small.tile([1, QG], F32, tag="dsb")
            nc.scalar.activation(d_sb[0:1, :], psn[D:D + 1, :], COPY_FN)
            d4 = small.tile([GT, P], F32, tag="d4")
            nc.scalar.dma_start(d4[:, :], d_sb[0:1, :])
            ps4 = psS.tile([P, GT], F32, tag="psS")
            nc.tensor.transpose(ps4[:, :], d4[:, :], ident4[:, :])
            recip = small.tile([P, GT], F32, tag="recip")
            nc.vector.reciprocal(recip[:, :], ps4[:, :])
            h = QG // 2
            nc.vector.tensor_copy(num_sb[0:D, g, 0:h], psn[0:D, 0:h])
            nc.vector.tensor_copy(num_sb[D:P, g, h:QG], psn[0:D, h:QG])
            return recip

        def out_self_pair(gp, t, og, eng_odd=None):
            """Out tiles t (rows 0:64) and t+2 (rows 64:128) as concurrent
            row-tiled matmuls; one FD-1024 evac per tile."""
            psA = psS.tile([P, 2, QG], F32, tag="psS")
            psB = psS.tile([P, 2, QG], F32, tag="psS")
            for eh in range(2):
                nc.tensor.matmul(psA[:, eh, :],
                                 num_sb[0:D, gp, t * P:(t + 1) * P],
                                 wv2_sb[0:D, eh * QG:(eh + 1) * QG],
                                 start=True, stop=True,
                                 tile_position=(0, 0), skip_group_check=True)
                nc.tensor.matmul(psB[:, eh, :],
                                 num_sb[D:P, gp, (t + 2) * P:(t + 3) * P],
                                 wv2_sb[D:P, eh * QG:(eh + 1) * QG],
                                 start=True, stop=True,
                                 tile_position=(D, 0), skip_group_check=True)
            psAf = psA.rearrange("p a b -> p (a b)")
            psBf = psB.rearrange("p a b -> p (a b)")
            ogf = og.rearrange("p t e -> p (t e)")
            nc.vector.tensor_scalar_mul(ogf[:, t * E:(t + 1) * E],
                                        psAf[:, :], recips[gp][:, t:t + 1])
            if eng_odd is nc.scalar:
                nc.scalar.activation(ogf[:, (t + 2) * E:(t + 3) * E],
                                     psBf[:, :], COPY_FN,
                                     scale=recips[gp][:, t + 2:t + 3])
            else:
                nc.vector.tensor_scalar_mul(ogf[:, (t + 2) * E:(t + 3) * E],
                                            psBf[:, :],
                                            recips[gp][:, t + 2:t + 3])

        # ---------------- schedule ----------------
        # pre-phase: consts, block 0 projections, strips(0)
        make_identity(nc, ident[:, :])
        make_identity(nc, ident4[:, :])
        make_upper_triangular(nc, tri[:, :], val=1.0, diag=True)
        qk_pass(0)
        vp_solo(0)
        vp_transp(0)
        entries = []
        entries += strip_pair(0, 0)
        entries += strip_pair(0, 2)

        recips = {}
        early = {"psn": None, "nxt": [], "emitted": 0}

        def num_mm(psn_t, j, pt_ap, lo, n_st):
            nc.tensor.matmul(
                psn_t[:, lo:QG], vp_sb[:, j, :], pt_ap,
                start=(j == 0), stop=(j == n_st - 1))

        for gi, g in enumerate(ORD):
            n_st = (g + 1) * GT
            gnext = ORD[gi + 1] if gi + 1 < NQG else None
            gprev = ORD[gi - 1] if gi >= 1 else None
            items = []
            og = None
            if gprev is not None:
                og = outp.tile([P, GT, E], BF16, tag="og")
            if gnext is not None:
                items.append(lambda ng=gnext: qk_pass(ng))
                if gi == 0:
                    items.append(lambda: vp_solo(1))
                    items.append(lambda: vp_transp(1))
                elif gi == 1:
                    items.append(lambda: vp_pair(2, 3))
                    items.append(lambda: vp_transp(2))
                elif gi == 2:
                    items.append(lambda: vp_transp(3))
            if gprev is not None:
                # late periods: ACT is exp-free, give it the B-half evacs
                oeng = nc.scalar if gi >= 2 else None
                for t in range(2):
                    items.append(
                        lambda t=t, gp=gprev, og=og, oeng=oeng:
                        out_self_pair(gp, t, og, eng_odd=oeng))
                def _ship(gp=gprev, og=og, gi=gi):
                    o_dst = out[gp * QG:(gp + 1) * QG, :]
                    eng = nc.gpsimd if gi == 2 else nc.sync
                    eng.dma_start(
                        o_dst.rearrange("(t p) e -> p t e", p=P), og[:, :, :])
                items.append(_ship)
            if gnext is not None:
                nxt = []
                if gi == 2:
                    # last group: emit its numerator matmuls inside this
                    # period, lagged 2 pairs behind its strips, so the
                    # final period collapses into the tail
                    early["nxt"] = nxt
                    psn_e = psN.tile([D + 1, QG], F32, tag="psn",
                                     name="psn_e")
                    early["psn"] = psn_e

                    def strip_and_num(j0, g2=gnext):
                        nxt.extend(strip_pair(g2, j0))
                        while early["emitted"] < len(nxt) - 4:
                            j, ap, lo = nxt[early["emitted"]]
                            num_mm(early["psn"], j, ap, lo, (g2 + 1) * GT)
                            early["emitted"] += 1
                    for j0 in range(0, (gnext + 1) * GT, 2):
                        items.append(lambda j0=j0: strip_and_num(j0))
                else:
                    for j0 in range(0, (gnext + 1) * GT, 2):
                        items.append(
                            lambda g2=gnext, j0=j0, acc=nxt:
                            acc.append(strip_pair(g2, j0)))
            if gi == NQG - 1:
                psn = early["psn"]
                # drain the lagged numerator matmuls of the last group
                while early["emitted"] < len(early["nxt"]):
                    j, ap, lo = early["nxt"][early["emitted"]]
                    num_mm(psn, j, ap, lo, n_st)
                    early["emitted"] += 1
                entries = []
            else:
                psn = psN.tile([D + 1, QG], F32, tag="psn")
            ii = 0
            for (j, pt_ap, lo) in entries:
                num_mm(psn, j, pt_ap, lo, n_st)
                if ii < len(items):
                    items[ii]()
                    ii += 1
            while ii < len(items):
                items[ii]()
                ii += 1
            recips[g] = epilogue(g, psn)
            if gnext is not None:
                entries = [e for pair in nxt for e in pair]

        # tail: last group's out tiles, split evac engines, per-tile DMA
        g = ORD[-1]
        og = outp.tile([P, GT, E], BF16, tag="og")
        tq = (nc.sync, nc.gpsimd, nc.scalar, nc.sync)
        for t in range(2):
            out_self_pair(g, t, og, eng_odd=nc.scalar)
            for tt in (t, t + 2):
                i = g * GT + tt
                tq[tt].dma_start(out[i * P:(i + 1) * P, :], og[:, tt, :])


_CACHE = {}


def _get_compiled():
    if "nc" not in _CACHE:
        nc = bacc.Bacc("TRN2", target_bir_lowering=False, debug=False,
                       enable_asserts=False, num_devices=B)
        build_kernel(nc)
        nc.compile()
        _CACHE["nc"] = nc
    return _CACHE["nc"]


def _tile_w(w):
    """[E, D] -> [128, NE, D] with w'[p, c, d] = w[c*128+p, d]."""
    return np.asarray(w, dtype=np.float32).reshape(NE, P, -1).transpose(1, 0, 2)


def _run(inputs, trace=False, tmpdir=None):
    nc = _get_compiled()
    bf16 = ml_dtypes.bfloat16
    x = np.asarray(inputs["x"], dtype=np.float32)
    wk_t = _tile_w(inputs["w_k"])                       # [128, NE, 64]
    wq_t = _tile_w(np.asarray(inputs["w_q"], dtype=np.float32) * SCALE)
    wqk_h = np.concatenate([wk_t, wq_t], axis=2)        # K cols 0:64, Q 64:128
    w = {
        "wqk": np.ascontiguousarray(
            wqk_h.reshape(P, NE * P).astype(bf16)),
        "w_v1": np.ascontiguousarray(
            _tile_w(inputs["w_v1"]).reshape(P, NE * D).astype(bf16)),
        "w_v2": np.ascontiguousarray(
            np.asarray(inputs["w_v2"], dtype=np.float32).astype(bf16)),
    }
    in_maps = [
        dict(x_t=np.ascontiguousarray(x[i].T.astype(bf16)), **w)
        for i in range(B)
    ]
    res = bass_utils.run_bass_kernel_spmd(
        nc, in_maps, core_ids=list(range(B)), trace=trace, tmpdir=tmpdir,
    )
    outs = np.stack([np.asarray(res.results[i]["out"]).astype(np.float32)
                     for i in range(B)])
    return outs, res


def kernel(**inputs) -> np.ndarray:
    outs, _ = _run(inputs, trace=False)
    return outs
